# revision 18
# baseline (speedup 1.0000x reference)
"""ChebNet (K=3, 3 layers) GNN on 8 Trainium2 NeuronCores.

Math: per layer, out = h@(W0-W2) + L(h@W1 + 2*L(h@W2)) + b, where
L(v) = -dis * S(dis * v), S = unweighted scatter-add over edges, and
dis = rsqrt(clamp(outdeg,1)) masked by outdeg>0.  The per-edge weight
w = dis[src]*dis[dst] folds into two dense per-node row scalings.

Sharding: nodes split into 8 contiguous ranges (one per core, 49 tiles of
128 rows).  Each core owns the edges whose dst falls in its range.  Per
core, dst tiles are PERMUTED into "slots" rank-matched by edge count
across cores so the shared (max-over-cores) chunk padding is tight; the
host un-permutes the output rows.

Each lap's operand is AllGathered in TWO halves aligned with the src-slot
split (slots 0..23 = window A rows, 24..48 = window B): gathers for
window A start as soon as AG-A lands, overlapping AG-B's transfer.  The
edge stream is processed window-major: all A gathers, A one-hot matmuls
accumulating in PSUM then staged to SBUF partials, then B gathers and
matmuls, then the epilogue.  Scatter-add runs on the tensor engine with
fp8e4 one-hot slabs (built on DVE in the first lap, streamed from DRAM
afterwards) against bf16 gathered rows.  dma_gather calls are ~2048 idxs
with single_packet=False (the 64-desc packet ceiling only allows 1024
idxs in single-packet mode) rotating over 4 SWDGE queues.
"""

import sys

sys.path.insert(0, "/opt/trn_rl_repo")

import numpy as np
from contextlib import ExitStack

_REAL = dict(N=50000, E=800000, NCORES=8, F0=128, F1=64, F2=16)


# ---------------------------------------------------------------- host prep
def _derive(cfg):
    c = dict(cfg)
    c["NPC"] = c["N"] // c["NCORES"]
    c["NT"] = -(-c["NPC"] // 128)
    c["NPAD"] = c["NT"] * 128
    c["NTA"] = c["NT"] // 2
    c["NTB"] = c["NT"] - c["NTA"]
    c["HA"] = c["NTA"] * 128
    c["HB"] = c["NTB"] * 128
    c["NGA"] = c["NCORES"] * c["HA"]
    c["NGB"] = c["NCORES"] * c["HB"]
    c["FW"] = 64  # lap working width (256B gather elements)
    assert c["NGA"] <= 32768 and c["NGB"] <= 32768
    assert c["N"] % c["NCORES"] == 0
    return c


def _prep(edge_index, c):
    N, E, NCORES = c["N"], c["E"], c["NCORES"]
    NPC, NT = c["NPC"], c["NT"]
    NTA, HA, HB = c["NTA"], c["HA"], c["HB"]

    src = np.asarray(edge_index[0], dtype=np.int64)
    dst = np.asarray(edge_index[1], dtype=np.int64)
    assert src.shape == (E,) and dst.shape == (E,)

    cs, ls = src // NPC, src % NPC
    cd, ld = dst // NPC, dst % NPC
    Ts, ps = ls >> 7, ls & 127
    Td, dl = ld >> 7, ld & 127

    # ---- per-core dst-tile -> slot permutation, rank-matched by edge count
    cnt_tot = np.zeros((NCORES, NT), np.int64)
    np.add.at(cnt_tot, (cd, Td), 1)
    perm = np.argsort(-cnt_tot, axis=1, kind="stable")  # slot s <- tile perm[c,s]
    inv_perm = np.empty_like(perm)
    for cc in range(NCORES):
        inv_perm[cc, perm[cc]] = np.arange(NT)

    ss = inv_perm[cs, Ts]  # src slot
    sd = inv_perm[cd, Td]  # dst slot
    win = (ss >= NTA).astype(np.int64)
    psrc = np.where(win == 0, cs * HA + ss * 128 + ps, cs * HB + (ss - NTA) * 128 + ps)

    # ---- chunk tables: edges grouped by (core, dst slot, window), src-sorted
    cnt = np.zeros((NCORES, NT, 2), np.int64)
    np.add.at(cnt, (cd, sd, win), 1)
    KA = np.maximum(1, -(-cnt[:, :, 0].max(0) // 128))
    KB = np.maximum(1, -(-cnt[:, :, 1].max(0) // 128))
    OFFA = np.concatenate([[0], np.cumsum(KA)]).astype(np.int64)
    OFFB = np.concatenate([[0], np.cumsum(KB)]).astype(np.int64)
    TLA, TLB = int(OFFA[-1]), int(OFFB[-1])

    order = np.lexsort((psrc, win, sd, cd))
    cd_s, sd_s, w_s = cd[order], sd[order], win[order]
    dl_s, psrc_s = dl[order], psrc[order]
    grp = (cd_s * NT + sd_s) * 2 + w_s
    gc = np.bincount(grp, minlength=NCORES * NT * 2)
    gstart = np.concatenate([[0], np.cumsum(gc)])[:-1]
    rank = np.arange(E) - gstart[grp]

    # one-hot slabs precomputed on host as fp8e4 bytes (1.0 = 0x38)
    gidxA = np.zeros((NCORES, TLA * 128), np.int16)
    gidxB = np.zeros((NCORES, TLB * 128), np.int16)
    ohA = np.zeros((NCORES, 128, TLA * 128), np.uint8)
    ohB = np.zeros((NCORES, 128, TLB * 128), np.uint8)
    for h, (gidx, oh, OFF) in enumerate([(gidxA, ohA, OFFA), (gidxB, ohB, OFFB)]):
        m = w_s == h
        slot = OFF[sd_s[m]] + rank[m] // 128
        part = rank[m] & 127
        gidx[cd_s[m], slot * 128 + part] = psrc_s[m].astype(np.int16)
        oh[cd_s[m], part, slot * 128 + dl_s[m]] = 0x38

    # ---- dis tables from host-side degrees, in slot order
    deg = np.bincount(src, minlength=N).astype(np.float64)
    dis_node = np.where(deg > 0, 1.0 / np.sqrt(np.maximum(deg, 1.0)), 0.0)
    dis_t = np.zeros((NCORES, 128, NT), np.float32)
    for cc in range(NCORES):
        for s in range(NT):
            T = perm[cc, s]
            lo = cc * NPC + T * 128
            nrow = min(128, NPC - T * 128)
            dis_t[cc, :nrow, s] = dis_node[lo : lo + nrow]

    def wrap(a):  # int16 [M*128] -> [128, M*8], idx j at [j%16, j//16], x8 replicated
        return np.tile(a.reshape(-1, 16).T, (8, 1)).copy()

    return dict(
        KA=KA, KB=KB, OFFA=OFFA, OFFB=OFFB, TLA=TLA, TLB=TLB,
        perm=perm,
        gidxA=[wrap(gidxA[cc]) for cc in range(NCORES)],
        gidxB=[wrap(gidxB[cc]) for cc in range(NCORES)],
        ohA=ohA, ohB=ohB,
        dis=dis_t, negdis=-dis_t, n2dis2=(-2.0 * dis_t * dis_t).astype(np.float32),
    )


# ---------------------------------------------------------------- device build
def _build(c, pp, Fins, use_bias):
    import concourse.bacc as bacc
    import concourse.tile as tile
    from concourse import mybir

    f32, i16 = mybir.dt.float32, mybir.dt.int16
    bf16, f8 = mybir.dt.bfloat16, mybir.dt.float8e4
    AOT = mybir.AluOpType
    NT, NTA, NTB = c["NT"], c["NTA"], c["NTB"]
    NPAD, HA, HB = c["NPAD"], c["HA"], c["HB"]
    NGA, NGB, FW = c["NGA"], c["NGB"], c["FW"]
    NCORES, F0, F2 = c["NCORES"], c["F0"], c["F2"]
    TLA, TLB = pp["TLA"], pp["TLB"]
    KA, KB = pp["KA"], pp["KB"]
    OFFA, OFFB = pp["OFFA"], pp["OFFB"]
    KMAX = int(max(KA.max(), KB.max()))
    NQ = 4
    CAP = 16  # chunks per dma_gather call (2048 idxs, multi-packet)
    GRP = 4
    groups = [list(range(g * GRP, min((g + 1) * GRP, NT))) for g in range(-(-NT // GRP))]
    NEV = 6  # AG events: (As, Cs) per layer
    # widest gather tile per window across groups
    NMAXA = max(int(OFFA[g[-1] + 1] - OFFA[g[0]]) for g in groups)
    NMAXB = max(int(OFFB[g[-1] + 1] - OFFB[g[0]]) for g in groups)

    nc = bacc.Bacc(num_devices=NCORES, num_swdge_queues=NQ)

    xin = nc.dram_tensor("x", [NPAD, F0], f32, kind="ExternalInput")
    gA_d = nc.dram_tensor("gidxA", [128, TLA * 8], i16, kind="ExternalInput")
    gB_d = nc.dram_tensor("gidxB", [128, TLB * 8], i16, kind="ExternalInput")
    dis_d = nc.dram_tensor("dis", [128, NT], f32, kind="ExternalInput")
    ndis_d = nc.dram_tensor("negdis", [128, NT], f32, kind="ExternalInput")
    n2d2_d = nc.dram_tensor("n2dis2", [128, NT], f32, kind="ExternalInput")
    id_d = nc.dram_tensor("ident", [128, 128], f32, kind="ExternalInput")
    W_d = {}
    for l in range(3):
        for nm in ("wa", "wb", "wc"):
            W_d[nm, l] = nc.dram_tensor(f"{nm}{l}", [Fins[l], FW], f32, kind="ExternalInput")
        if use_bias[l]:
            W_d["br", l] = nc.dram_tensor(f"br{l}", [128, FW], f32, kind="ExternalInput")
    y_d = nc.dram_tensor("y", [NPAD, F2], f32, kind="ExternalOutput")

    agiA = [nc.dram_tensor(f"agiA{i}", [HA, FW], f32) for i in range(NEV)]
    agiB = [nc.dram_tensor(f"agiB{i}", [HB, FW], f32) for i in range(NEV)]
    agoA = [nc.dram_tensor(f"agoA{i}", [NGA, FW], f32, addr_space="Shared") for i in range(NEV)]
    agoB = [nc.dram_tensor(f"agoB{i}", [NGB, FW], f32, addr_space="Shared") for i in range(NEV)]
    ohA_d = nc.dram_tensor("ohA", [128, TLA * 128], f8, kind="ExternalInput")
    ohB_d = nc.dram_tensor("ohB", [128, TLB * 128], f8, kind="ExternalInput")

    xv = xin.rearrange("(t p) f -> p t f", p=128)
    yv = y_d.rearrange("(t p) f -> p t f", p=128)
    agiA_v = [t.rearrange("(t p) f -> p t f", p=128) for t in agiA]
    agiB_v = [t.rearrange("(t p) f -> p t f", p=128) for t in agiB]

    with tile.TileContext(nc) as tc, ExitStack() as ctx:
        cst = ctx.enter_context(tc.tile_pool(name="cst", bufs=1))
        big = ctx.enter_context(tc.tile_pool(name="big", bufs=1))
        gp = ctx.enter_context(tc.tile_pool(name="gp", bufs=3))
        gpb = ctx.enter_context(tc.tile_pool(name="gpb", bufs=4))
        ohp = ctx.enter_context(tc.tile_pool(name="ohp", bufs=2 * GRP))
        smp = ctx.enter_context(tc.tile_pool(name="smp", bufs=4))
        slb = ctx.enter_context(tc.tile_pool(name="slb", bufs=2))
        psA = ctx.enter_context(tc.tile_pool(name="psA", bufs=4, space="PSUM"))
        psT = ctx.enter_context(tc.tile_pool(name="psT", bufs=2, space="PSUM"))
        psD = ctx.enter_context(tc.tile_pool(name="psD", bufs=2, space="PSUM"))

        # constants
        ident_f = cst.tile([128, 128], f32)
        nc.sync.dma_start(ident_f[:], id_d[:])
        gA = cst.tile([128, TLA * 8], i16)
        nc.sync.dma_start(gA[:], gA_d[:])
        gB = cst.tile([128, TLB * 8], i16)
        nc.sync.dma_start(gB[:], gB_d[:])
        dis = cst.tile([128, NT], f32)
        nc.sync.dma_start(dis[:], dis_d[:])
        negdis = cst.tile([128, NT], f32)
        nc.sync.dma_start(negdis[:], ndis_d[:])
        n2dis2 = cst.tile([128, NT], f32)
        nc.sync.dma_start(n2dis2[:], n2d2_d[:])
        Wt = {}
        Wb = {}
        for k, d in W_d.items():
            if k[0] == "br":
                Wt[k] = cst.tile([128, FW], f32, name=f"w_{k[0]}_{k[1]}", tag=f"w_{k[0]}_{k[1]}")
                nc.sync.dma_start(Wt[k][: d.shape[0], :], d[:])
            else:
                wstage = smp.tile([128, FW], f32, name=f"ws_{k[0]}_{k[1]}", tag="wstage")
                nc.sync.dma_start(wstage[: d.shape[0], :], d[:])
                Wb[k] = cst.tile([128, FW], bf16, name=f"wb_{k[0]}_{k[1]}", tag=f"wb_{k[0]}_{k[1]}")
                nc.scalar.copy(Wb[k][: d.shape[0], :], wstage[: d.shape[0], :])
        ident_b = cst.tile([128, 128], bf16)
        nc.scalar.copy(ident_b[:], ident_f[:])

        qctr = [0]

        def emit_gathers(tl, winname, ev):
            """Issue dma_gather calls for window A/B of slot group tl; returns
            the f32 gather tile and the group's chunk base."""
            if winname == "A":
                OFF, src, gidx, nmax, tag = OFFA, agoA[ev][:, :], gA, NMAXA, "gA"
            else:
                OFF, src, gidx, nmax, tag = OFFB, agoB[ev][:, :], gB, NMAXB, "gB"
            a, b = int(OFF[tl[0]]), int(OFF[tl[-1] + 1])
            n = b - a
            g = gp.tile([128, nmax, FW], f32, tag=tag, name=f"g_{tag}_{tl[0]}")
            ncalls = -(-n // CAP)
            szs = [n // ncalls + (1 if i < n % ncalls else 0) for i in range(ncalls)]
            o = 0
            for sz in szs:
                nc.gpsimd.dma_gather(
                    g[:, o : o + sz, :], src,
                    gidx[:, (a + o) * 8 : (a + o + sz) * 8],
                    num_idxs=sz * 128, num_idxs_reg=sz * 128, elem_size=FW,
                    queue_num=qctr[0] % NQ, single_packet=False,
                )
                qctr[0] += 1
                o += sz
            return g, a, n

        def prefetch_slabs(tl, winname):
            """Load the host-built fp8 one-hot slabs for a slot group on the
            Act HWDGE queue, one group ahead of the matmuls that use them."""
            if winname == "A":
                OFF, K, oh_d, tags = OFFA, KA, ohA_d, "slabA"
            else:
                OFF, K, oh_d, tags = OFFB, KB, ohB_d, "slabB"
            out = []
            for t in tl:
                k0, kk = int(OFF[t]), int(K[t])
                slab = ohp.tile([128, KMAX * 128], f8, tag=tags, name=f"slab_{tags}_{t}")
                nc.scalar.dma_start(
                    slab[:, : kk * 128], oh_d[:, k0 * 128 : (k0 + kk) * 128]
                )
                out.append(slab)
            return out

        def compute_phase(tl, winname, slabs, g, a, PA, epi):
            """bf16-convert gathered rows and run the scatter matmuls.
            Window A stages its PSUM sum to PA; window B folds PA back in
            with an identity matmul (f32) and runs the epilogue."""
            OFF, K = (OFFA, KA) if winname == "A" else (OFFB, KB)
            for t, slab in zip(tl, slabs):
                k0, kk = int(OFF[t]), int(K[t])
                gb = gpb.tile([128, KMAX, FW], bf16, tag="gb", name=f"gb_{t}")
                nc.vector.tensor_copy(gb[:, :kk, :], g[:, k0 - a : k0 - a + kk, :])
                acc = psA.tile([128, FW], f32, tag="acc", name=f"acc_{t}")
                if winname == "B":
                    nc.tensor.matmul(
                        acc[:], ident_f[:], PA[:, t, :], start=True, stop=False
                    )
                for k in range(kk):
                    nc.tensor.matmul(
                        acc[:], slab[:, k * 128 : (k + 1) * 128],
                        gb[:, k, :],
                        start=(winname == "A" and k == 0),
                        stop=(winname == "B" and k == kk - 1),
                    )
                if winname == "A":
                    nc.scalar.copy(PA[:, t, :], acc[:])
                else:
                    epi(t, acc)

        def sub_ag(ev, half):
            if half == "A":
                ins, outs = agiA[ev], agoA[ev]
            else:
                ins, outs = agiB[ev], agoB[ev]
            nc.gpsimd.collective_compute(
                "AllGather", mybir.AluOpType.bypass,
                replica_groups=[list(range(NCORES))],
                ins=[ins[:, :]], outs=[outs[:, :]],
            )

        def lap(ev, epi, hookA=None, hookB=None, chase=None, preB=None):
            # gather/compute interleave per group so pool WAR deps stay
            # backward-looking; the gpsimd stream is still all window-A
            # calls, then all window-B calls, then the next AG triggers.
            # `chase(gi, tl)` runs after each B group's epilogues (used to
            # emit the next layer's pass-1 work behind this lap).
            PA = big.tile([128, NT, FW], f32, tag="PA", name=f"PA_{ev}")
            for winname, do_epi in (("A", False), ("B", True)):
                slabs = prefetch_slabs(groups[0], winname)
                for gi, tl in enumerate(groups):
                    g, a, n = emit_gathers(tl, winname, ev)
                    cur, slabs = slabs, (
                        prefetch_slabs(groups[gi + 1], winname)
                        if gi + 1 < len(groups) else None
                    )
                    compute_phase(tl, winname, cur, g, a, PA, epi)
                    if not do_epi and gi == 2 and preB is not None:
                        preB()
                        preB = None
                    if do_epi:
                        if chase is not None:
                            chase(gi, tl)
                        if hookA is not None and tl[0] <= NTA - 1 <= tl[-1]:
                            hookA()
                            hookA = None
            if hookA is not None:
                hookA()
            if hookB is not None:
                hookB()

        # ---------------- layer building blocks
        def pass1_group(l, tl, As, hT_all, hsT_all, h_prev):
            """Transposes + As tiles (the AG input) for one slot group."""
            Fin = Fins[l]
            w = len(tl) * 128
            c0 = tl[0] * 128
            for t in tl:
                if l == 0:
                    ht = smp.tile([128, F0], f32, tag="xt")
                    nc.sync.dma_start(ht[:], xv[:, t, :])
                    ht_b = smp.tile([128, F0], bf16, tag="xtb")
                    nc.scalar.copy(ht_b[:], ht[:])
                    ht_ap = ht_b[:]
                else:
                    ht_ap = h_prev[:, t, :]
                ps = psT.tile([128, 128], bf16, tag="pt")
                nc.tensor.transpose(ps[:Fin, :], ht_ap, ident_b[:])
                nc.scalar.copy(hT_all[:Fin, (t * 128) : (t + 1) * 128], ps[:Fin, :])
                hs = smp.tile([128, Fin], bf16, tag="hs")
                nc.scalar.mul(hs[:], ht_ap, dis[:, t : t + 1])
                ps2 = psT.tile([128, 128], bf16, tag="pt")
                nc.tensor.transpose(ps2[:Fin, :], hs[:], ident_b[:])
                nc.scalar.copy(hsT_all[:Fin, (t * 128) : (t + 1) * 128], ps2[:Fin, :])
            pd = psD.tile([64, GRP * 128], f32, tag="pd")
            nc.tensor.matmul(pd[:, :w], Wb["wc", l][:Fin, :], hsT_all[:Fin, c0 : c0 + w])
            pT = slb.tile([64, GRP * 128], bf16, tag="pT")
            nc.scalar.copy(pT[:, :w], pd[:, :w])
            for u, t in enumerate(tl):
                pb = psT.tile([128, 128], bf16, tag="pt")
                nc.tensor.transpose(
                    pb[:, :FW], pT[:FW, u * 128 : (u + 1) * 128], ident_b[:FW, :FW]
                )
                nc.scalar.copy(As[:, t, :], pb[:, :FW])

        def pass2(l, Cs1, Oa, hT_all, hsT_all):
            Fin = Fins[l]
            for tl in groups:
                w = len(tl) * 128
                c0 = tl[0] * 128
                for dstbuf, wkey, srcT in (
                    (Cs1, ("wb", l), hsT_all),
                    (Oa, ("wa", l), hT_all),
                ):
                    pd = psD.tile([64, GRP * 128], f32, tag="pd")
                    nc.tensor.matmul(pd[:, :w], Wb[wkey][:Fin, :], srcT[:Fin, c0 : c0 + w])
                    pT = slb.tile([64, GRP * 128], bf16, tag="pT")
                    nc.scalar.copy(pT[:, :w], pd[:, :w])
                    for u, t in enumerate(tl):
                        pb = psT.tile([128, 128], bf16, tag="pt")
                        nc.tensor.transpose(
                            pb[:, :FW], pT[:FW, u * 128 : (u + 1) * 128], ident_b[:FW, :FW]
                        )
                        nc.scalar.copy(dstbuf[:, t, :], pb[:, :FW])

        # ---------------- layers
        h_prev = None
        hT_all = big.tile([128, NT * 128], bf16, tag="hTa")
        hsT_all = big.tile([128, NT * 128], bf16, tag="hsTa")
        As = big.tile([128, NT, FW], f32, tag="AsCs")
        for tl in groups:
            pass1_group(0, tl, As, hT_all, hsT_all, None)
            if tl[0] <= NTA - 1 <= tl[-1]:
                nc.sync.dma_start(agiA_v[0][:, :, :], As[:, :NTA, :])
                sub_ag(0, "A")
        nc.sync.dma_start(agiB_v[0][:, :, :], As[:, NTA:, :])
        sub_ag(0, "B")

        preB_carry = [None]
        for l in range(3):
            evA, evC = 2 * l, 2 * l + 1
            # As was fully consumed (DMA'd to ag_in) before the first lap's
            # epilogues write Cs, so they share one buffer.
            Cs1 = big.tile([128, NT, FW], f32, tag="Cs1")
            Oa = big.tile([128, NT, FW], f32, tag="Oa")
            pass2(l, Cs1, Oa, hT_all, hsT_all)

            Cs = big.tile([128, NT, FW], f32, tag="AsCs")

            def epi1(t, acc):
                tmp2 = smp.tile([128, FW], f32, tag="t2")
                nc.vector.tensor_scalar_mul(tmp2[:], acc[:], n2dis2[:, t : t + 1])
                nc.vector.tensor_add(Cs[:, t, :], Cs1[:, t, :], tmp2[:])

            def hook_CA():
                nc.sync.dma_start(agiA_v[evC][:, :, :], Cs[:, :NTA, :])
                sub_ag(evC, "A")

            def pre_CB():
                nc.sync.dma_start(agiB_v[evC][:, :, :], Cs[:, NTA:, :])
                sub_ag(evC, "B")

            lap(evA, epi=epi1, hookA=hook_CA, preB=preB_carry[0])

            hn = (
                big.tile([128, NT, FW], bf16, tag=f"h{l}", name=f"h{l}")
                if l < 2 else None
            )

            def epi2(t, acc):
                tmp2 = smp.tile([128, FW], f32, tag="t2")
                nc.vector.tensor_scalar_mul(tmp2[:], acc[:], negdis[:, t : t + 1])
                if use_bias[l]:
                    tmp3 = smp.tile([128, FW], f32, tag="t3")
                    nc.vector.tensor_add(tmp3[:], tmp2[:], Oa[:, t, :])
                    pre = smp.tile([128, FW], f32, tag="t4")
                    nc.vector.tensor_add(pre[:], tmp3[:], Wt["br", l][:, :])
                else:
                    pre = smp.tile([128, FW], f32, tag="t3")
                    nc.vector.tensor_add(pre[:], tmp2[:], Oa[:, t, :])
                if l < 2:
                    nc.vector.tensor_scalar_max(hn[:, t, :], pre[:], 0.0)
                else:
                    yt = smp.tile([128, FW], f32, tag="yt")
                    nc.vector.tensor_copy(yt[:], pre[:])
                    nc.sync.dma_start(yv[:, t, :], yt[:, :F2])

            if l < 2:
                # chase: emit the next layer's pass-1 work behind this lap's
                # B-phase so its AG can fire as soon as the tiles are ready.
                As_n = big.tile([128, NT, FW], f32, tag="AsCs")

                def chase(gi, tl, l=l, As_n=As_n, hn=hn):
                    pass1_group(l + 1, tl, As_n, hT_all, hsT_all, hn)
                    if tl[0] <= NTA - 1 <= tl[-1]:
                        nc.sync.dma_start(agiA_v[2 * l + 2][:, :, :], As_n[:, :NTA, :])
                        sub_ag(2 * l + 2, "A")

                def pre_AB(l=l, As_n=As_n):
                    nc.sync.dma_start(agiB_v[2 * l + 2][:, :, :], As_n[:, NTA:, :])
                    sub_ag(2 * l + 2, "B")

                lap(evC, epi=epi2, chase=chase, preB=pre_CB)
                preB_carry[0] = pre_AB
            else:
                lap(evC, epi=epi2, preB=pre_CB)
            h_prev = hn

    nc.compile()
    return nc


# ---------------------------------------------------------------- entry
def _run(x, edge_index, Ws, bs, cfg=None, trace=False):
    from concourse.bass_utils import run_bass_kernel_spmd

    c = _derive(cfg or _REAL)
    N, NCORES, NPC, NPAD = c["N"], c["NCORES"], c["NPC"], c["NPAD"]
    NT, F0, F2, FW = c["NT"], c["F0"], c["F2"], c["FW"]

    x = np.ascontiguousarray(np.asarray(x, dtype=np.float32))
    pp = _prep(edge_index, c)
    perm = pp["perm"]

    Fins = [F0, c["F1"], c["F1"]]
    use_bias = [bool(np.any(b)) for b in bs]
    nc = _build(c, pp, Fins, use_bias)

    import ml_dtypes
    ident = np.eye(128, dtype=np.float32)

    def padW(w, fin):
        out = np.zeros((fin, FW), np.float32)
        out[: w.shape[0], : w.shape[1]] = w
        return out

    base = {"ident": ident}
    for l in range(3):
        W = np.asarray(Ws[l], dtype=np.float32)
        base[f"wa{l}"] = padW(W[0] - W[2], Fins[l])
        base[f"wb{l}"] = padW(W[1], Fins[l])
        base[f"wc{l}"] = padW(W[2], Fins[l])
        if use_bias[l]:
            br = np.zeros((128, FW), np.float32)
            br[:, : bs[l].shape[0]] = np.asarray(bs[l], np.float32)
            base[f"br{l}"] = br

    in_maps = []
    for cc in range(NCORES):
        xl = np.zeros((NPAD, F0), np.float32)
        for s in range(NT):
            T = perm[cc, s]
            nrow = min(128, NPC - T * 128)
            xl[s * 128 : s * 128 + nrow] = x[cc * NPC + T * 128 : cc * NPC + T * 128 + nrow]
        in_maps.append(
            dict(
                base,
                x=xl,
                gidxA=pp["gidxA"][cc],
                gidxB=pp["gidxB"][cc],
                ohA=pp["ohA"][cc].view(ml_dtypes.float8_e4m3fn),
                ohB=pp["ohB"][cc].view(ml_dtypes.float8_e4m3fn),
                dis=np.ascontiguousarray(pp["dis"][cc]),
                negdis=np.ascontiguousarray(pp["negdis"][cc]),
                n2dis2=np.ascontiguousarray(pp["n2dis2"][cc]),
            )
        )

    res = run_bass_kernel_spmd(nc, in_maps, core_ids=list(range(NCORES)), trace=trace)
    out = np.empty((N, F2), np.float32)
    for cc in range(NCORES):
        yl = res.results[cc]["y"]
        for s in range(NT):
            T = perm[cc, s]
            nrow = min(128, NPC - T * 128)
            out[cc * NPC + T * 128 : cc * NPC + T * 128 + nrow] = yl[s * 128 : s * 128 + nrow]
    return out[:, :F2], res


def kernel(x, edge_index, W1, b1, Wm, bm, W2, b2):
    out, _ = _run(
        np.asarray(x), np.asarray(edge_index),
        [np.asarray(W1), np.asarray(Wm), np.asarray(W2)],
        [np.asarray(b1), np.asarray(bm), np.asarray(b2)],
    )
    return out
